# revision 24
# baseline (speedup 1.0000x reference)
"""Trainium2 Bass kernel for nn_Crosscoder (batch top-k crosscoder).

Feature-sharded over 8 NeuronCores (2048 features each). Per core:
fp32 encode (z_pre^T = W_enc^T (x - b_pre)^T + b_enc) in f-major layout,
exact global batch-top-k threshold on device (segmented max8 -> per-
partition top-136 -> AllGather -> replicated bisection), bf16 decode,
chunked ReduceScatter of partial x_hat^T, per-layer SSE on device.
"""
import sys
sys.path.insert(0, "/opt/trn_rl_repo")
import numpy as np

import concourse.bacc as bacc
import concourse.mybir as mybir
import concourse.tile as tile
from concourse.bass_utils import run_bass_kernel_spmd

B, L, D, F = 1024, 4, 768, 16384
LD = L * D                    # 3072
NCORES = 8
FS = F // NCORES              # 2048 features per core
NKC = LD // 128               # 24 contraction chunks (encode)
NFT = FS // 128               # 16 feature tiles per core
NLT = LD // 128               # 24 ld tiles (decode output)
NFC = FS // 128               # 16 feature chunks (decode contraction)
NCHUNK = 4                    # ReduceScatter chunks
LT_PER_CHUNK = NLT // NCHUNK
NSL = LT_PER_CHUNK * 128 // NCORES   # rows per core per chunk (96)
SEG = 128                     # L1 max8 segment width (batch cols)
TOPM = 104                    # per-partition candidates kept
NROUNDS = TOPM // 8
BISECT_ITERS = 25
HI_INIT = 4.0

f32 = mybir.dt.float32
bf16 = mybir.dt.bfloat16
f32r = mybir.dt.float32r


def _build(R_total):
    nc = bacc.Bacc("TRN2", target_bir_lowering=False)

    xT_ext = nc.dram_tensor("xT", [NKC, 128, B], f32, kind="ExternalInput")
    bpre_ext = nc.dram_tensor("bpre", [LD], f32, kind="ExternalInput")
    wenc_ext = nc.dram_tensor("wenc", [NFT, NKC, 128, 128], f32, kind="ExternalInput")
    benc_ext = nc.dram_tensor("benc", [FS], f32, kind="ExternalInput")
    wdec_ext = nc.dram_tensor("wdec", [NLT, NFC, 128, 128], f32, kind="ExternalInput")
    bdec_ext = nc.dram_tensor("bdec_s", [NCHUNK, NSL], f32, kind="ExternalInput")
    xs_ext = nc.dram_tensor("x_s", [NCHUNK, NSL, B], f32, kind="ExternalInput")

    zpre_ext = nc.dram_tensor("z_pre_s", [FS, B], f32, kind="ExternalOutput")
    z_ext = nc.dram_tensor("z_s", [FS, B], f32, kind="ExternalOutput")
    xhat_ext = nc.dram_tensor("xhat_s", [NCHUNK, NSL, B], f32, kind="ExternalOutput")
    sse_ext = nc.dram_tensor("sse_s", [NCHUNK, NSL], f32, kind="ExternalOutput")
    tau_ext = nc.dram_tensor("tau", [1, 1], f32, kind="ExternalOutput")

    grp = [list(range(NCORES))]
    n_cand_total = 128 * NCORES * TOPM
    s_thresh = float(2 * R_total - n_cand_total)

    with tile.TileContext(nc) as tc:
        with tc.tile_pool(name="cpool", bufs=1) as cpool, \
             tc.tile_pool(name="psE", bufs=4, space="PSUM") as psE, \
             tc.tile_pool(name="psS", bufs=2, space="PSUM") as psS, \
             tc.tile_pool(name="dram", bufs=1, space="DRAM") as dram:

            bpre_sb = cpool.tile([128, NKC], f32, tag="bpre")
            nc.scalar.dma_start(out=bpre_sb[:], in_=bpre_ext.rearrange("(nk p) -> p nk", p=128))
            benc_sb = cpool.tile([128, NFT], f32, tag="benc")
            nc.scalar.dma_start(out=benc_sb[:], in_=benc_ext.rearrange("(ft p) -> p ft", p=128))

            # ====== Phase 1: encode, f32r hi/lo 3-term split, 2 halves ======
            work = cpool.tile([128, NFT * 32], f32, tag="work")
            with tc.tile_pool(name="xpool", bufs=1) as xpool, \
                 tc.tile_pool(name="xkpool", bufs=3) as xkpool, \
                 tc.tile_pool(name="wspool", bufs=2) as wspool, \
                 tc.tile_pool(name="wepool", bufs=2) as wepool, \
                 tc.tile_pool(name="stpool", bufs=4) as stpool:
                for bh in range(2):
                    bsl = slice(bh * 512, (bh + 1) * 512)
                    xh_h = xpool.tile([128, NKC, 512], f32r, tag="xh")
                    xl_h = xpool.tile([128, NKC, 512], f32r, tag="xl")
                    # prefetch + cast ft0 weights ahead of the x-prep chain
                    wenc_p = wspool.tile([128, NKC, 128], f32, tag="wenc")
                    nc.sync.dma_start(out=wenc_p[:],
                                      in_=wenc_ext[0].rearrange("nk p m -> p nk m"))
                    wh_p = wepool.tile([128, NKC, 128], f32r, tag="wh")
                    wl_p = wepool.tile([128, NKC, 128], f32r, tag="wl")
                    nc.scalar.copy(wh_p[:], wenc_p[:])
                    nc.vector.tensor_tensor(out=wl_p[:], in0=wenc_p[:], in1=wh_p[:],
                                            op=mybir.AluOpType.subtract)
                    for kc in range(NKC):
                        xkc = xkpool.tile([128, 512], f32, tag="xkc")
                        nc.scalar.dma_start(out=xkc[:], in_=xT_ext[kc, :, bsl])
                        nc.vector.tensor_scalar_sub(xkc[:], xkc[:],
                                                    bpre_sb[:, kc:kc + 1])
                        nc.scalar.copy(xh_h[:, kc, :], xkc[:])
                        nc.vector.tensor_tensor(out=xl_h[:, kc, :], in0=xkc[:],
                                                in1=xh_h[:, kc, :],
                                                op=mybir.AluOpType.subtract)
                    for ft in range(NFT):
                        if ft == 0:
                            wh_t, wl_t = wh_p, wl_p
                        else:
                            wenc_t = wspool.tile([128, NKC, 128], f32, tag="wenc")
                            nc.sync.dma_start(out=wenc_t[:],
                                              in_=wenc_ext[ft].rearrange("nk p m -> p nk m"))
                            wh_t = wepool.tile([128, NKC, 128], f32r, tag="wh")
                            wl_t = wepool.tile([128, NKC, 128], f32r, tag="wl")
                            nc.scalar.copy(wh_t[:], wenc_t[:])
                            nc.vector.tensor_tensor(out=wl_t[:], in0=wenc_t[:],
                                                    in1=wh_t[:],
                                                    op=mybir.AluOpType.subtract)
                        ps = psE.tile([128, 512], f32, tag="psE")
                        for kc in range(NKC):
                            nc.tensor.matmul(ps[:], wh_t[:, kc, :], xh_h[:, kc, :],
                                             start=(kc == 0), stop=False)
                            nc.tensor.matmul(ps[:], wl_t[:, kc, :], xh_h[:, kc, :],
                                             start=False, stop=False)
                            nc.tensor.matmul(ps[:], wh_t[:, kc, :], xl_h[:, kc, :],
                                             start=False, stop=(kc == NKC - 1))
                        stage = stpool.tile([128, 512], f32, tag="st")
                        nc.vector.tensor_scalar_add(stage[:], ps[:], benc_sb[:, ft:ft + 1])
                        nc.scalar.dma_start(out=zpre_ext[ft * 128:(ft + 1) * 128, bsl],
                                          in_=stage[:])
                        for sg in range(2):
                            o = (ft * 4 + bh * 2 + sg) * 8
                            nc.vector.max(work[:, o:o + 8],
                                          stage[:, sg * 256:(sg + 1) * 256])

            # ================= Phase 2: global threshold ====================
            p2pool_cm = tc.tile_pool(name="p2pool", bufs=1)
            p2pool = p2pool_cm.__enter__()
            cand = p2pool.tile([128, TOPM], f32, tag="cand")
            for rd in range(NROUNDS):
                nc.vector.max(cand[:, rd * 8:(rd + 1) * 8], work[:])
                if rd < NROUNDS - 1:
                    nc.vector.match_replace(work[:], cand[:, rd * 8:(rd + 1) * 8],
                                            work[:], -1e30)

            ag_in = dram.tile([128, TOPM], f32, tag="agi")
            ag_out = dram.tile([NCORES * 128, TOPM], f32, tag="ago", addr_space="Shared")
            nc.sync.dma_start(out=ag_in[:], in_=cand[:])
            nc.gpsimd.collective_compute(
                "AllGather", mybir.AluOpType.bypass, replica_groups=grp,
                ins=[ag_in.opt()], outs=[ag_out.opt()])
            gath = p2pool.tile([128, NCORES, TOPM], f32, tag="gath")
            for c in range(NCORES):
                nc.sync.dma_start(out=gath[:, c, :], in_=ag_out[c * 128:(c + 1) * 128, :])

            ones_row = p2pool.tile([1, 128], f32, tag="ones_row")
            nc.vector.memset(ones_row[:], 1.0)
            ones_col = p2pool.tile([128, 1], f32, tag="ones_col")
            nc.vector.memset(ones_col[:], 1.0)
            neg_col = p2pool.tile([128, 1], f32, tag="neg_col")
            nc.vector.memset(neg_col[:], -1.0)
            lo_v = p2pool.tile([128, 1], f32, tag="lo_v")
            nlo_v = p2pool.tile([128, 1], f32, tag="nlo_v")
            w_v = p2pool.tile([128, 1], f32, tag="w_v")
            nm_v = p2pool.tile([128, 1], f32, tag="nm_v")
            t_v = p2pool.tile([128, 1], f32, tag="t_v")
            b_v = p2pool.tile([128, 1], f32, tag="b_v")
            acc_v = p2pool.tile([128, 1], f32, tag="acc_v")
            s_sb = p2pool.tile([1, 1], f32, tag="s_sb")
            junk = p2pool.tile([128, NCORES * TOPM], f32, tag="junk")
            nc.vector.memset(lo_v[:], 0.0)
            nc.vector.memset(w_v[:], HI_INIT)

            # Phase-3 pools opened early so zpre/wdec prefetch + casts can
            # overlap the bisection.
            with tc.tile_pool(name="zprepool", bufs=8) as zprepool, \
                 tc.tile_pool(name="wdpool", bufs=4) as wdpool, \
                 tc.tile_pool(name="spool", bufs=4) as spool, \
                 tc.tile_pool(name="zpool", bufs=1) as zpool:
                zpre_tiles = []
                for ft in range(NFT):
                    zpre_t = zprepool.tile([128, B], f32, tag="zpre")
                    nc.sync.dma_start(out=zpre_t[:],
                                      in_=zpre_ext[ft * 128:(ft + 1) * 128, :])
                    zpre_tiles.append(zpre_t)

                for it in range(BISECT_ITERS):
                    nc.vector.tensor_scalar_mul(w_v[:], w_v[:], 0.5)
                    # nm = -(lo + w)
                    nc.vector.scalar_tensor_tensor(
                        out=nm_v[:], in0=lo_v[:], scalar=w_v[:, 0:1], in1=neg_col[:],
                        op0=mybir.AluOpType.add, op1=mybir.AluOpType.mult)
                    nc.scalar.activation(junk[:], gath[:],
                                         mybir.ActivationFunctionType.Sign,
                                         bias=nm_v[:], scale=1.0, accum_out=acc_v[:])
                    ps_s = psS.tile([1, 1], f32, tag="psS1")
                    nc.tensor.matmul(ps_s[:], acc_v[:], ones_col[:], start=True, stop=True)
                    nc.vector.tensor_copy(s_sb[:], ps_s[:])
                    ps_b = psS.tile([128, 1], f32, tag="psSb")
                    nc.tensor.matmul(ps_b[:], ones_row[:], s_sb[:], start=True, stop=True)
                    nc.vector.tensor_scalar(out=b_v[:], in0=ps_b[:], scalar1=s_thresh,
                                            scalar2=None, op0=mybir.AluOpType.is_ge)
                    # lo += b * w
                    nc.vector.scalar_tensor_tensor(
                        out=lo_v[:], in0=b_v[:], scalar=w_v[:, 0:1], in1=lo_v[:],
                        op0=mybir.AluOpType.mult, op1=mybir.AluOpType.add)
                nc.sync.dma_start(out=tau_ext[:], in_=lo_v[0:1, :])

                # ---- mask + cast (z = zpre * (zpre > lo)) ----
                zbf = zpool.tile([128, NFC, B], f32r, tag="zbf")
                for ft in range(NFT):
                    z_t = spool.tile([128, B], f32, tag="fin")
                    nc.vector.scalar_tensor_tensor(
                        out=z_t[:], in0=zpre_tiles[ft][:], scalar=lo_v[:, 0:1],
                        in1=zpre_tiles[ft][:],
                        op0=mybir.AluOpType.is_gt, op1=mybir.AluOpType.mult)
                    nc.sync.dma_start(out=z_ext[ft * 128:(ft + 1) * 128, :], in_=z_t[:])
                    nc.scalar.copy(zbf[:, ft, :], z_t[:])

                # ---- decode (bf16) + chunked ReduceScatter ----
                bdec_sb = p2pool.tile([NSL, NCHUNK], f32, tag="bdec")
                nc.gpsimd.dma_start(out=bdec_sb[:], in_=bdec_ext.rearrange("t p -> p t"))
                sse_sb = p2pool.tile([NSL, NCHUNK], f32, tag="sse")
                xhatp = [dram.tile([LT_PER_CHUNK * 128, B], f32, tag=f"xhatp{j}", name=f"xhatp{j}")
                         for j in range(NCHUNK)]
                rs_out = [dram.tile([NSL, B], f32, tag=f"rso{j}", name=f"rso{j}")
                          for j in range(NCHUNK)]
                for j in range(NCHUNK):
                    for li in range(LT_PER_CHUNK):
                        lt = j * LT_PER_CHUNK + li
                        wdec_t = wdpool.tile([128, NFC, 128], f32, tag="wdec")
                        nc.sync.dma_start(out=wdec_t[:],
                                          in_=wdec_ext[lt].rearrange("fc p m -> p fc m"))
                        wdec_bf = wdpool.tile([128, NFC, 128], f32r, tag="wdecbf")
                        nc.scalar.copy(wdec_bf[:], wdec_t[:])
                        pd0 = psE.tile([128, 512], f32, tag="psE")
                        pd1 = psE.tile([128, 512], f32, tag="psE")
                        for fc in range(NFC):
                            st, sp = (fc == 0), (fc == NFC - 1)
                            nc.tensor.matmul(pd0[:], wdec_bf[:, fc, :],
                                             zbf[:, fc, 0:512], start=st, stop=sp)
                            nc.tensor.matmul(pd1[:], wdec_bf[:, fc, :],
                                             zbf[:, fc, 512:1024], start=st, stop=sp)
                        xh_t = spool.tile([128, B], f32, tag="xh")
                        nc.scalar.copy(xh_t[:, 0:512], pd0[:])
                        nc.scalar.copy(xh_t[:, 512:1024], pd1[:])
                        nc.scalar.dma_start(out=xhatp[j][li * 128:(li + 1) * 128, :],
                                          in_=xh_t[:])
                    nc.gpsimd.collective_compute(
                        "ReduceScatter", mybir.AluOpType.add, replica_groups=grp,
                        ins=[xhatp[j].opt()], outs=[rs_out[j].opt()])
                    h_t = spool.tile([NSL, B], f32, tag="fin")
                    nc.gpsimd.dma_start(out=h_t[:], in_=rs_out[j][:])
                    nc.vector.tensor_scalar_add(h_t[:], h_t[:], bdec_sb[:, j:j + 1])
                    nc.gpsimd.dma_start(out=xhat_ext[j], in_=h_t[:])
                    xs_t = spool.tile([NSL, B], f32, tag="fin")
                    nc.gpsimd.dma_start(out=xs_t[:], in_=xs_ext[j])
                    d_t = spool.tile([NSL, B], f32, tag="fin")
                    nc.vector.tensor_tensor(out=d_t[:], in0=xs_t[:], in1=h_t[:],
                                            op=mybir.AluOpType.subtract)
                    sq_t = spool.tile([NSL, B], f32, tag="fin")
                    nc.scalar.activation(sq_t[:], d_t[:],
                                         mybir.ActivationFunctionType.Square,
                                         bias=0.0, scale=1.0,
                                         accum_out=sse_sb[:, j:j + 1])
                nc.gpsimd.dma_start(out=sse_ext.rearrange("t p -> p t"), in_=sse_sb[:])
            p2pool_cm.__exit__(None, None, None)

    nc.compile()
    return nc


_CACHE = {}


def _get_program(R_total):
    if R_total not in _CACHE:
        _CACHE[R_total] = _build(R_total)
    return _CACHE[R_total]


def _core_rows(c):
    """Global x_hat^T row-blocks (start, NSL) owned by core c after the
    chunked reduce-scatter: chunk j covers rows [768j, 768(j+1))."""
    return [LT_PER_CHUNK * 128 * j + NSL * c for j in range(NCHUNK)]


def _make_in_maps(x, b_pre, W_enc, b_enc, W_dec, b_dec):
    x_flat = x.reshape(B, LD)
    xT = np.ascontiguousarray(x_flat.T).reshape(NKC, 128, B)
    wdec_r = np.ascontiguousarray(W_dec.transpose(1, 0, 2).reshape(F, LD))
    bdec_flat = b_dec.reshape(LD)
    xT_rows = xT.reshape(LD, B)

    in_maps = []
    for c in range(NCORES):
        wenc_c = W_enc[:, c * FS:(c + 1) * FS]
        wenc_t = np.ascontiguousarray(
            wenc_c.reshape(NKC, 128, NFT, 128).transpose(2, 0, 1, 3))
        wdec_c = wdec_r[c * FS:(c + 1) * FS]
        wdec_t = np.ascontiguousarray(
            wdec_c.reshape(NFC, 128, NLT, 128).transpose(2, 0, 1, 3))
        rows = _core_rows(c)
        bdec_s = np.stack([bdec_flat[r:r + NSL] for r in rows])
        x_s = np.stack([xT_rows[r:r + NSL] for r in rows])
        in_maps.append({
            "xT": xT,
            "bpre": b_pre,
            "wenc": wenc_t,
            "benc": np.ascontiguousarray(b_enc[c * FS:(c + 1) * FS]),
            "wdec": wdec_t,
            "bdec_s": np.ascontiguousarray(bdec_s),
            "x_s": np.ascontiguousarray(x_s),
        })
    return in_maps


def kernel(x, b_pre, W_enc, b_enc, W_dec, b_dec, k):
    x = np.asarray(x, dtype=np.float32)
    b_pre = np.asarray(b_pre, dtype=np.float32)
    W_enc = np.asarray(W_enc, dtype=np.float32)
    b_enc = np.asarray(b_enc, dtype=np.float32)
    W_dec = np.asarray(W_dec, dtype=np.float32)
    b_dec = np.asarray(b_dec, dtype=np.float32)
    k_val = int(k)
    R_total = min(B * k_val, B * F)

    nc = _get_program(R_total)
    in_maps = _make_in_maps(x, b_pre, W_enc, b_enc, W_dec, b_dec)
    res = run_bass_kernel_spmd(nc, in_maps, list(range(NCORES)))
    outs = res.results

    z_pre = np.empty((B, F), dtype=np.float32)
    z = np.empty((B, F), dtype=np.float32)
    xhatT = np.empty((LD, B), dtype=np.float32)
    sse = np.zeros(LD, dtype=np.float64)
    for c in range(NCORES):
        z_pre[:, c * FS:(c + 1) * FS] = outs[c]["z_pre_s"].T
        z[:, c * FS:(c + 1) * FS] = outs[c]["z_s"].T
        for j, r in enumerate(_core_rows(c)):
            xhatT[r:r + NSL] = outs[c]["xhat_s"][j]
            sse[r:r + NSL] = outs[c]["sse_s"][j]

    x_hat = np.ascontiguousarray(xhatT.T).reshape(B, L, D)
    per_layer_mse = (sse.reshape(L, D).sum(axis=1) / (B * D)).astype(np.float32)
    loss = np.float32(per_layer_mse.sum())
    return x_hat, z, z_pre, loss, per_layer_mse
